# revision 7
# baseline (speedup 1.0000x reference)
"""DLRM forward (26-table embedding lookup + dot interaction + MLPs) on 8 trn2 cores.

Strategy: pure data-parallel batch sharding (1024 samples/core), embedding
tables replicated in each core's HBM, no collectives.  bf16 compute with f32
PSUM accumulation.  Per core:
  - indirect-DMA gather of 26*1024 embedding rows (bf16, 128 rows/descriptor-set)
  - bottom MLP in batch-on-free layout -> xT [64, 1024]
  - PE transposes of gathered tiles -> TT [64(d), s, 27(feat)] slabs
  - per-sample Gram matmuls (stationary = TT[s], 27x27 out), 16 samples packed
    per PSUM bank with sample-minor column layout
  - Z fed to top MLP via symmetrized 729-row weight (host-built), with a
    partition-expanding SBUF->SBUF DMA building [108, B] K-tiles (2KB runs)
  - top MLP -> sigmoid -> [1, 1024] f32 out per core
"""

import sys

sys.path.insert(0, "/opt/trn_rl_repo")

import numpy as np
import ml_dtypes

import concourse.bass as bass
import concourse.mybir as mybir
import concourse.tile as tile
from concourse import bacc
from concourse import bass_utils
from concourse.bass_interp import get_hw_module
from concourse.masks import make_identity

BF16 = mybir.dt.bfloat16
F32 = mybir.dt.float32
I32 = mybir.dt.int32

N_TABLES = 26
N_FEAT = 27
D = 64
N_DENSE = 13
N_CORES = 8

_LI, _LJ = np.tril_indices(N_FEAT, k=-1)  # 351 pairs, matches reference order


def build(vocab, bc, nc_obj=None):
    """Build the per-core Bass graph. bc = per-core batch (multiple of 128)."""
    P = 128
    T = bc // P
    NZK = 7  # ceil(27/4) groups of 4 i-rows -> K-tiles of <=108
    nc = nc_obj or bacc.Bacc("TRN2", target_bir_lowering=False, debug=False,
                             num_devices=N_CORES)

    # ---- DRAM tensors (names are the in_map keys) ----
    d_wemb = nc.dram_tensor("wemb", [N_TABLES * vocab, D], BF16, kind="ExternalInput")
    d_idx = nc.dram_tensor("idx", [P, T * N_TABLES], I32, kind="ExternalInput")
    d_xt13 = nc.dram_tensor("xt13", [N_DENSE, bc], BF16, kind="ExternalInput")
    d_bw0t = nc.dram_tensor("bw0t", [N_DENSE, 512], BF16, kind="ExternalInput")
    d_bb0 = nc.dram_tensor("bb0c", [128, 4], F32, kind="ExternalInput")
    d_bw1t = nc.dram_tensor("bw1t", [128, 4, 256], BF16, kind="ExternalInput")
    d_bb1 = nc.dram_tensor("bb1c", [128, 2], F32, kind="ExternalInput")
    d_bw2t = nc.dram_tensor("bw2t", [128, 2, 64], BF16, kind="ExternalInput")
    d_bb2 = nc.dram_tensor("bb2c", [64, 1], F32, kind="ExternalInput")
    d_tw0xt = nc.dram_tensor("tw0xt", [D, 512], BF16, kind="ExternalInput")
    d_wzt = nc.dram_tensor("wzt", [108, NZK, 512], BF16, kind="ExternalInput")
    d_tb0 = nc.dram_tensor("tb0c", [128, 4], F32, kind="ExternalInput")
    d_tw1t = nc.dram_tensor("tw1t", [128, 4, 256], BF16, kind="ExternalInput")
    d_tb1 = nc.dram_tensor("tb1c", [128, 2], F32, kind="ExternalInput")
    d_tw2t = nc.dram_tensor("tw2t", [128, 2, 1], BF16, kind="ExternalInput")
    d_tb2 = nc.dram_tensor("tb2c", [1, 1], F32, kind="ExternalInput")
    d_out = nc.dram_tensor("out", [1, bc], F32, kind="ExternalOutput")

    AF = mybir.ActivationFunctionType
    NH = bc // 512  # number of 512-wide N slices
    assert bc % 512 == 0 or bc == 128

    def nsl(j):  # j-th N slice (512 wide, or bc if smaller)
        w = min(512, bc)
        return slice(j * w, (j + 1) * w)

    NHN = max(1, bc // 512)

    with tile.TileContext(nc) as tc:
        with (
            tc.tile_pool(name="singles", bufs=1) as singles,
            tc.tile_pool(name="gpool", bufs=3) as gpool,
            tc.tile_pool(name="pmm", bufs=3, space="PSUM") as pmm,
            tc.tile_pool(name="ptp", bufs=2, space="PSUM") as ptp,
            tc.tile_pool(name="pz", bufs=2, space="PSUM") as pz,
        ):
            # ---- load constants ----
            sb_idx = singles.tile([P, T * N_TABLES], I32)
            nc.sync.dma_start(out=sb_idx[:], in_=d_idx[:])
            sb_xt13 = singles.tile([N_DENSE, bc], BF16)
            nc.sync.dma_start(out=sb_xt13[:], in_=d_xt13[:])
            sb_bw0t = singles.tile([N_DENSE, 512], BF16)
            nc.sync.dma_start(out=sb_bw0t[:], in_=d_bw0t[:])
            sb_bb0 = singles.tile([128, 4], F32)
            nc.sync.dma_start(out=sb_bb0[:], in_=d_bb0[:])
            sb_bw1t = singles.tile([128, 4, 256], BF16)
            nc.sync.dma_start(out=sb_bw1t[:], in_=d_bw1t[:])
            sb_bb1 = singles.tile([128, 2], F32)
            nc.sync.dma_start(out=sb_bb1[:], in_=d_bb1[:])
            sb_bw2t = singles.tile([128, 2, 64], BF16)
            nc.sync.dma_start(out=sb_bw2t[:], in_=d_bw2t[:])
            sb_bb2 = singles.tile([64, 1], F32)
            nc.sync.dma_start(out=sb_bb2[:], in_=d_bb2[:])
            sb_tw0xt = singles.tile([D, 512], BF16)
            nc.sync.dma_start(out=sb_tw0xt[:], in_=d_tw0xt[:])
            sb_wzt = singles.tile([108, NZK, 512], BF16)
            nc.sync.dma_start(out=sb_wzt[:], in_=d_wzt[:])
            sb_tb0 = singles.tile([128, 4], F32)
            nc.sync.dma_start(out=sb_tb0[:], in_=d_tb0[:])
            sb_tw1t = singles.tile([128, 4, 256], BF16)
            nc.sync.dma_start(out=sb_tw1t[:], in_=d_tw1t[:])
            sb_tb1 = singles.tile([128, 2], F32)
            nc.sync.dma_start(out=sb_tb1[:], in_=d_tb1[:])
            sb_tw2t = singles.tile([128, 2, 1], BF16)
            nc.sync.dma_start(out=sb_tw2t[:], in_=d_tw2t[:])
            sb_tb2 = singles.tile([1, 1], F32)
            nc.sync.dma_start(out=sb_tb2[:], in_=d_tb2[:])

            sb_ident = singles.tile([P, P], BF16)
            make_identity(nc, sb_ident[:])

            sb_xt64 = singles.tile([D, bc], BF16)
            # [d, feature, sample] — t-major so transpose evacs write contiguous
            sb_tts = [singles.tile([D, N_FEAT, P], BF16, tag=f"tt{k}",
                                   name=f"sb_tt{k}")
                      for k in range(T)]
            sb_zg = singles.tile([N_FEAT, N_FEAT, bc], BF16)
            sb_zk = singles.tile([108, NZK, bc], BF16)
            sb_h1 = singles.tile([128, 4, bc], BF16)
            sb_h2 = singles.tile([128, 2, bc], BF16)
            sb_g1 = singles.tile([128, 4, bc], BF16)
            sb_g2 = singles.tile([128, 2, bc], BF16)
            sb_out = singles.tile([1, bc], F32)

            # zero-pad rows of the last ZK tile (weights there are zero too,
            # but keep the data finite)
            nc.vector.memset(sb_zk[:, NZK - 1, :], 0.0)

            # ---- gather + transpose per 128-sample tile ----
            for k in range(T):
                g = gpool.tile([P, N_TABLES * D], BF16, tag="g")
                for t in range(N_TABLES):
                    nc.gpsimd.indirect_dma_start(
                        out=g[:, t * D:(t + 1) * D],
                        out_offset=None,
                        in_=d_wemb[:],
                        in_offset=bass.IndirectOffsetOnAxis(
                            ap=sb_idx[:, k * N_TABLES + t: k * N_TABLES + t + 1],
                            axis=0,
                        ),
                    )
                for t in range(N_TABLES):
                    tp = ptp.tile([D, P], BF16, tag="tp")
                    nc.tensor.transpose(tp[:], g[:, t * D:(t + 1) * D], sb_ident[:])
                    nc.vector.tensor_copy(out=sb_tts[k][:, 1 + t, :], in_=tp[:])

            # ---- bottom MLP (batch-on-free) ----
            for m in range(4):
                for j in range(NHN):
                    ps = pmm.tile([128, 512], F32, tag="ps")
                    nc.tensor.matmul(ps[:, :min(512, bc)],
                                     sb_bw0t[:, m * 128:(m + 1) * 128],
                                     sb_xt13[:, nsl(j)], start=True, stop=True)
                    nc.scalar.activation(sb_h1[:, m, nsl(j)], ps[:, :min(512, bc)],
                                         AF.Relu, bias=sb_bb0[:, m:m + 1])
            for m in range(2):
                for j in range(NHN):
                    ps = pmm.tile([128, 512], F32, tag="ps")
                    for kk in range(4):
                        nc.tensor.matmul(ps[:, :min(512, bc)],
                                         sb_bw1t[:, kk, m * 128:(m + 1) * 128],
                                         sb_h1[:, kk, nsl(j)],
                                         start=(kk == 0), stop=(kk == 3))
                    nc.scalar.activation(sb_h2[:, m, nsl(j)], ps[:, :min(512, bc)],
                                         AF.Relu, bias=sb_bb1[:, m:m + 1])
            for j in range(NHN):
                ps = pmm.tile([128, 512], F32, tag="ps")
                for kk in range(2):
                    nc.tensor.matmul(ps[:64, :min(512, bc)], sb_bw2t[:, kk, :],
                                     sb_h2[:, kk, nsl(j)],
                                     start=(kk == 0), stop=(kk == 1))
                nc.scalar.activation(sb_xt64[:, nsl(j)], ps[:64, :min(512, bc)],
                                     AF.Relu, bias=sb_bb2[:, :1])

            # x is feature 0 of the interaction: copy into TT slabs
            for k in range(T):
                nc.vector.tensor_copy(out=sb_tts[k][:, 0, :],
                                      in_=sb_xt64[:, k * P:(k + 1) * P])

            # ---- per-sample Gram matmuls ----
            for k in range(T):
                for sg in range(8):  # 16 samples per PSUM bank, sample-major
                    psz = pz.tile([N_FEAT, 16, N_FEAT], F32, tag="psz")
                    for si in range(16):
                        st = sb_tts[k][:, :, sg * 16 + si]
                        nc.tensor.matmul(psz[:, si, :], st, st,
                                         start=True, stop=True)
                    # [27(i), s, j] -> zg [27(i), j, s] (strided both sides)
                    nc.scalar.activation(
                        sb_zg[:, :, k * P + sg * 16: k * P + sg * 16 + 16]
                        .rearrange("p j s -> p s j"),
                        psz[:], AF.Copy)

            # ---- repartition Z: [27(i) part, 27(j), s] -> [(4i,27j) part, s] ----
            for gi in range(NZK):
                ni = min(4, N_FEAT - 4 * gi)
                nc.sync.dma_start(out=sb_zk[:ni * 27, gi, :],
                                  in_=sb_zg[4 * gi: 4 * gi + ni, :, :])

            # ---- top MLP ----
            for m in range(4):
                for j in range(NHN):
                    ps = pmm.tile([128, 512], F32, tag="ps")
                    nc.tensor.matmul(ps[:, :min(512, bc)],
                                     sb_tw0xt[:, m * 128:(m + 1) * 128],
                                     sb_xt64[:, nsl(j)], start=True, stop=False)
                    for gi in range(NZK):
                        nc.tensor.matmul(ps[:, :min(512, bc)],
                                         sb_wzt[:, gi, m * 128:(m + 1) * 128],
                                         sb_zk[:, gi, nsl(j)],
                                         start=False, stop=(gi == NZK - 1))
                    nc.scalar.activation(sb_g1[:, m, nsl(j)], ps[:, :min(512, bc)],
                                         AF.Relu, bias=sb_tb0[:, m:m + 1])
            for m in range(2):
                for j in range(NHN):
                    ps = pmm.tile([128, 512], F32, tag="ps")
                    for kk in range(4):
                        nc.tensor.matmul(ps[:, :min(512, bc)],
                                         sb_tw1t[:, kk, m * 128:(m + 1) * 128],
                                         sb_g1[:, kk, nsl(j)],
                                         start=(kk == 0), stop=(kk == 3))
                    nc.scalar.activation(sb_g2[:, m, nsl(j)], ps[:, :min(512, bc)],
                                         AF.Relu, bias=sb_tb1[:, m:m + 1])
            for j in range(NHN):
                ps = pmm.tile([128, 512], F32, tag="ps")
                for kk in range(2):
                    nc.tensor.matmul(ps[:1, :min(512, bc)], sb_tw2t[:, kk, :],
                                     sb_g2[:, kk, nsl(j)],
                                     start=(kk == 0), stop=(kk == 1))
                nc.scalar.activation(sb_out[:, nsl(j)], ps[:1, :min(512, bc)],
                                     AF.Sigmoid, bias=sb_tb2[:, :1])

            nc.sync.dma_start(out=d_out[:], in_=sb_out[:])

    nc.compile()
    return nc


def prepare_inputs(vocab, bc, dense_x, lS_i, Wemb,
                   bW0, bb0, bW1, bb1, bW2, bb2,
                   tW0, tb0, tW1, tb1, tW2, tb2):
    """Host-side prep: shard + lay out numpy arrays exactly as the SBUF wants."""
    bf = ml_dtypes.bfloat16
    ncores = dense_x.shape[0] // bc
    wemb = np.ascontiguousarray(Wemb.reshape(N_TABLES * vocab, D)).astype(bf)

    # symmetrized top-L0 interaction weight: [108, 7, 512]
    wz = np.zeros((N_FEAT, N_FEAT, 512), np.float32)
    for p in range(len(_LI)):
        wz[_LI[p], _LJ[p], :] = 0.5 * tW0[:, D + p]
        wz[_LJ[p], _LI[p], :] = 0.5 * tW0[:, D + p]
    NZK = 7
    wzt = np.zeros((108, NZK, 512), np.float32)
    for gi in range(NZK):
        ni = min(4, N_FEAT - 4 * gi)
        blk = wz[4 * gi:4 * gi + ni].reshape(ni * 27, 512)
        wzt[:ni * 27, gi, :] = blk

    shared = dict(
        wemb=wemb,
        bw0t=np.ascontiguousarray(bW0.T).astype(bf),
        bb0c=np.ascontiguousarray(bb0.reshape(4, 128).T).astype(np.float32),
        bw1t=np.ascontiguousarray(bW1.T.reshape(4, 128, 256).transpose(1, 0, 2)).astype(bf),
        bb1c=np.ascontiguousarray(bb1.reshape(2, 128).T).astype(np.float32),
        bw2t=np.ascontiguousarray(bW2.T.reshape(2, 128, 64).transpose(1, 0, 2)).astype(bf),
        bb2c=np.ascontiguousarray(bb2.reshape(64, 1)).astype(np.float32),
        tw0xt=np.ascontiguousarray(tW0[:, :D].T).astype(bf),
        wzt=wzt.astype(bf),
        tb0c=np.ascontiguousarray(tb0.reshape(4, 128).T).astype(np.float32),
        tw1t=np.ascontiguousarray(tW1.T.reshape(4, 128, 256).transpose(1, 0, 2)).astype(bf),
        tb1c=np.ascontiguousarray(tb1.reshape(2, 128).T).astype(np.float32),
        tw2t=np.ascontiguousarray(tW2.T.reshape(2, 128, 1).transpose(1, 0, 2)).astype(bf),
        tb2c=np.array([[tb2[0]]], np.float32),
    )

    # flat row ids: table t's rows live at [t*vocab, (t+1)*vocab)
    flat_idx = (lS_i.astype(np.int64)
                + (np.arange(N_TABLES, dtype=np.int64) * vocab)[:, None])
    flat_idx = flat_idx.astype(np.int32)  # [26, B]

    in_maps = []
    T = bc // 128
    for c in range(ncores):
        sl = slice(c * bc, (c + 1) * bc)
        # idx[p, k*26+t] = flat_idx[t, c*bc + k*128 + p]
        blk = flat_idx[:, sl].reshape(N_TABLES, T, 128)  # [t, k, p]
        idx = np.ascontiguousarray(blk.transpose(2, 1, 0).reshape(128, T * N_TABLES))
        in_maps.append(dict(
            shared,
            idx=idx,
            xt13=np.ascontiguousarray(dense_x[sl].T).astype(bf),
        ))
    return in_maps


_CACHED = {}


def kernel(dense_x, lS_i, lS_o, Wemb,
           bW0, bb0, bW1, bb1, bW2, bb2,
           tW0, tb0, tW1, tb1, tW2, tb2):
    del lS_o  # offsets are arange(B): one index per bag
    vocab = Wemb.shape[1]
    B = dense_x.shape[0]
    bc = B // N_CORES
    in_maps = prepare_inputs(vocab, bc, np.asarray(dense_x), np.asarray(lS_i),
                             np.asarray(Wemb),
                             np.asarray(bW0), np.asarray(bb0),
                             np.asarray(bW1), np.asarray(bb1),
                             np.asarray(bW2), np.asarray(bb2),
                             np.asarray(tW0), np.asarray(tb0),
                             np.asarray(tW1), np.asarray(tb1),
                             np.asarray(tW2), np.asarray(tb2))
    key = (vocab, bc)
    if key not in _CACHED:
        nc = build(vocab, bc)
        m = nc.m
        nc.m = get_hw_module(m)
        _CACHED[key] = nc
    nc = _CACHED[key]
    res = bass_utils.run_bass_kernel_spmd(nc, in_maps,
                                          core_ids=list(range(N_CORES)))
    out = np.concatenate([r["out"] for r in res.results], axis=1).T  # [B, 1]
    return np.ascontiguousarray(out.astype(np.float32))


# revision 9
# speedup vs baseline: 1.1952x; 1.1952x over previous
"""DLRM forward (26-table embedding lookup + dot interaction + MLPs) on 8 trn2 cores.

Strategy: pure data-parallel batch sharding (1024 samples/core), embedding
tables replicated in each core's HBM, no collectives.  bf16 compute with f32
PSUM accumulation.  Per core:
  - indirect-DMA gather of 26*1024 embedding rows (bf16, 128 rows/descriptor-set)
  - bottom MLP in batch-on-free layout -> xT [64, 1024]
  - PE transposes of gathered tiles -> TT [64(d), s, 27(feat)] slabs
  - per-sample Gram matmuls (stationary = TT[s], 27x27 out), 16 samples packed
    per PSUM bank with sample-minor column layout
  - Z fed to top MLP via symmetrized 729-row weight (host-built), with a
    partition-expanding SBUF->SBUF DMA building [108, B] K-tiles (2KB runs)
  - top MLP -> sigmoid -> [1, 1024] f32 out per core
"""

import sys

sys.path.insert(0, "/opt/trn_rl_repo")

import numpy as np
import ml_dtypes

import concourse.bass as bass
import concourse.mybir as mybir
import concourse.tile as tile
from concourse import bacc
from concourse import bass_utils
from concourse.bass_interp import get_hw_module
from concourse.masks import make_identity

BF16 = mybir.dt.bfloat16
F32 = mybir.dt.float32
I32 = mybir.dt.int32

N_TABLES = 26
N_FEAT = 27
D = 64
N_DENSE = 13
N_CORES = 8

_LI, _LJ = np.tril_indices(N_FEAT, k=-1)  # 351 pairs, matches reference order


def build(vocab, bc, nc_obj=None):
    """Build the per-core Bass graph. bc = per-core batch (multiple of 128)."""
    P = 128
    T = bc // P
    NZK = 7  # ceil(27/4) groups of 4 i-rows -> K-tiles of <=108
    nc = nc_obj or bacc.Bacc("TRN2", target_bir_lowering=False, debug=False,
                             num_devices=N_CORES)

    # ---- DRAM tensors (names are the in_map keys) ----
    d_wemb = nc.dram_tensor("wemb", [N_TABLES * vocab, D], BF16, kind="ExternalInput")
    d_idx = nc.dram_tensor("idx", [P, T * N_TABLES], I32, kind="ExternalInput")
    d_xt13 = nc.dram_tensor("xt13", [N_DENSE, bc], BF16, kind="ExternalInput")
    d_bw0t = nc.dram_tensor("bw0t", [N_DENSE, 512], BF16, kind="ExternalInput")
    d_bb0 = nc.dram_tensor("bb0c", [128, 4], F32, kind="ExternalInput")
    d_bw1t = nc.dram_tensor("bw1t", [128, 4, 256], BF16, kind="ExternalInput")
    d_bb1 = nc.dram_tensor("bb1c", [128, 2], F32, kind="ExternalInput")
    d_bw2t = nc.dram_tensor("bw2t", [128, 2, 64], BF16, kind="ExternalInput")
    d_bb2 = nc.dram_tensor("bb2c", [64, 1], F32, kind="ExternalInput")
    d_tw0xt = nc.dram_tensor("tw0xt", [D, 512], BF16, kind="ExternalInput")
    d_wzt = nc.dram_tensor("wzt", [108, NZK, 512], BF16, kind="ExternalInput")
    d_tb0 = nc.dram_tensor("tb0c", [128, 4], F32, kind="ExternalInput")
    d_tw1t = nc.dram_tensor("tw1t", [128, 4, 256], BF16, kind="ExternalInput")
    d_tb1 = nc.dram_tensor("tb1c", [128, 2], F32, kind="ExternalInput")
    d_tw2t = nc.dram_tensor("tw2t", [128, 2, 1], BF16, kind="ExternalInput")
    d_tb2 = nc.dram_tensor("tb2c", [1, 1], F32, kind="ExternalInput")
    d_out = nc.dram_tensor("out", [1, bc], F32, kind="ExternalOutput")

    AF = mybir.ActivationFunctionType
    NH = bc // 512  # number of 512-wide N slices
    assert bc % 512 == 0 or bc == 128

    def nsl(j):  # j-th N slice (512 wide, or bc if smaller)
        w = min(512, bc)
        return slice(j * w, (j + 1) * w)

    NHN = max(1, bc // 512)

    with tile.TileContext(nc) as tc:
        with (
            tc.tile_pool(name="singles", bufs=1) as singles,
            tc.tile_pool(name="gpool", bufs=3) as gpool,
            tc.tile_pool(name="pmm", bufs=3, space="PSUM") as pmm,
            tc.tile_pool(name="ptp", bufs=2, space="PSUM") as ptp,
            tc.tile_pool(name="pz", bufs=2, space="PSUM") as pz,
        ):
            # ---- load constants ----
            sb_idx = singles.tile([P, T * N_TABLES], I32)
            nc.sync.dma_start(out=sb_idx[:], in_=d_idx[:])
            sb_xt13 = singles.tile([N_DENSE, bc], BF16)
            nc.sync.dma_start(out=sb_xt13[:], in_=d_xt13[:])
            sb_bw0t = singles.tile([N_DENSE, 512], BF16)
            nc.sync.dma_start(out=sb_bw0t[:], in_=d_bw0t[:])
            sb_bb0 = singles.tile([128, 4], F32)
            nc.sync.dma_start(out=sb_bb0[:], in_=d_bb0[:])
            sb_bw1t = singles.tile([128, 4, 256], BF16)
            nc.sync.dma_start(out=sb_bw1t[:], in_=d_bw1t[:])
            sb_bb1 = singles.tile([128, 2], F32)
            nc.sync.dma_start(out=sb_bb1[:], in_=d_bb1[:])
            sb_bw2t = singles.tile([128, 2, 64], BF16)
            nc.sync.dma_start(out=sb_bw2t[:], in_=d_bw2t[:])
            sb_bb2 = singles.tile([64, 1], F32)
            nc.sync.dma_start(out=sb_bb2[:], in_=d_bb2[:])
            sb_tw0xt = singles.tile([D, 512], BF16)
            nc.sync.dma_start(out=sb_tw0xt[:], in_=d_tw0xt[:])
            sb_wzt = singles.tile([108, NZK, 512], BF16)
            nc.sync.dma_start(out=sb_wzt[:], in_=d_wzt[:])
            sb_tb0 = singles.tile([128, 4], F32)
            nc.sync.dma_start(out=sb_tb0[:], in_=d_tb0[:])
            sb_tw1t = singles.tile([128, 4, 256], BF16)
            nc.sync.dma_start(out=sb_tw1t[:], in_=d_tw1t[:])
            sb_tb1 = singles.tile([128, 2], F32)
            nc.sync.dma_start(out=sb_tb1[:], in_=d_tb1[:])
            sb_tw2t = singles.tile([128, 2, 1], BF16)
            nc.sync.dma_start(out=sb_tw2t[:], in_=d_tw2t[:])
            sb_tb2 = singles.tile([1, 1], F32)
            nc.sync.dma_start(out=sb_tb2[:], in_=d_tb2[:])

            sb_ident = singles.tile([P, P], BF16)
            make_identity(nc, sb_ident[:])

            sb_xt64 = singles.tile([D, bc], BF16)
            # [d, feature, sample] — t-major so transpose evacs write contiguous
            sb_tts = [singles.tile([D, N_FEAT, P], BF16, tag=f"tt{k}",
                                   name=f"sb_tt{k}")
                      for k in range(T)]
            sb_zg = singles.tile([N_FEAT, N_FEAT, bc], BF16)
            sb_zk = singles.tile([108, NZK, bc], BF16)
            sb_h1 = singles.tile([128, 4, bc], BF16)
            sb_h2 = singles.tile([128, 2, bc], BF16)
            sb_g1 = singles.tile([128, 4, bc], BF16)
            sb_g2 = singles.tile([128, 2, bc], BF16)
            sb_out = singles.tile([1, bc], F32)

            # zero-pad rows of the last ZK tile (weights there are zero too,
            # but keep the data finite)
            nc.vector.memset(sb_zk[:, NZK - 1, :], 0.0)

            # ---- gather + transpose per 128-sample tile ----
            for k in range(T):
                g = gpool.tile([P, N_TABLES, D], BF16, tag="g")
                nc.gpsimd.indirect_dma_start(
                    out=g[:],
                    out_offset=None,
                    in_=d_wemb[:],
                    in_offset=bass.IndirectOffsetOnAxis(
                        ap=sb_idx[:, k * N_TABLES:(k + 1) * N_TABLES],
                        axis=0,
                    ),
                )
                for t in range(N_TABLES):
                    tp = ptp.tile([D, P], BF16, tag="tp")
                    nc.tensor.transpose(tp[:], g[:, t, :], sb_ident[:])
                    nc.vector.tensor_copy(out=sb_tts[k][:, 1 + t, :], in_=tp[:])

            # ---- bottom MLP (batch-on-free) ----
            for m in range(4):
                for j in range(NHN):
                    ps = pmm.tile([128, 512], F32, tag="ps")
                    nc.tensor.matmul(ps[:, :min(512, bc)],
                                     sb_bw0t[:, m * 128:(m + 1) * 128],
                                     sb_xt13[:, nsl(j)], start=True, stop=True)
                    nc.scalar.activation(sb_h1[:, m, nsl(j)], ps[:, :min(512, bc)],
                                         AF.Relu, bias=sb_bb0[:, m:m + 1])
            for m in range(2):
                for j in range(NHN):
                    ps = pmm.tile([128, 512], F32, tag="ps")
                    for kk in range(4):
                        nc.tensor.matmul(ps[:, :min(512, bc)],
                                         sb_bw1t[:, kk, m * 128:(m + 1) * 128],
                                         sb_h1[:, kk, nsl(j)],
                                         start=(kk == 0), stop=(kk == 3))
                    nc.scalar.activation(sb_h2[:, m, nsl(j)], ps[:, :min(512, bc)],
                                         AF.Relu, bias=sb_bb1[:, m:m + 1])
            for j in range(NHN):
                ps = pmm.tile([128, 512], F32, tag="ps")
                for kk in range(2):
                    nc.tensor.matmul(ps[:64, :min(512, bc)], sb_bw2t[:, kk, :],
                                     sb_h2[:, kk, nsl(j)],
                                     start=(kk == 0), stop=(kk == 1))
                nc.scalar.activation(sb_xt64[:, nsl(j)], ps[:64, :min(512, bc)],
                                     AF.Relu, bias=sb_bb2[:, :1])

            # x is feature 0 of the interaction: copy into TT slabs
            for k in range(T):
                nc.vector.tensor_copy(out=sb_tts[k][:, 0, :],
                                      in_=sb_xt64[:, k * P:(k + 1) * P])

            # ---- per-sample Gram matmuls ----
            for k in range(T):
                for sg in range(8):  # 16 samples per PSUM bank, sample-major
                    psz = pz.tile([N_FEAT, 16, N_FEAT], F32, tag="psz")
                    for si in range(16):
                        st = sb_tts[k][:, :, sg * 16 + si]
                        nc.tensor.matmul(psz[:, si, :], st, st,
                                         start=True, stop=True)
                    # [27(i), s, j] -> zg [27(i), j, s] (strided both sides)
                    nc.scalar.activation(
                        sb_zg[:, :, k * P + sg * 16: k * P + sg * 16 + 16]
                        .rearrange("p j s -> p s j"),
                        psz[:], AF.Copy)

            # ---- repartition Z: [27(i) part, 27(j), s] -> [(4i,27j) part, s] ----
            for gi in range(NZK):
                ni = min(4, N_FEAT - 4 * gi)
                nc.sync.dma_start(out=sb_zk[:ni * 27, gi, :],
                                  in_=sb_zg[4 * gi: 4 * gi + ni, :, :])

            # ---- top MLP ----
            for m in range(4):
                for j in range(NHN):
                    ps = pmm.tile([128, 512], F32, tag="ps")
                    nc.tensor.matmul(ps[:, :min(512, bc)],
                                     sb_tw0xt[:, m * 128:(m + 1) * 128],
                                     sb_xt64[:, nsl(j)], start=True, stop=False)
                    for gi in range(NZK):
                        nc.tensor.matmul(ps[:, :min(512, bc)],
                                         sb_wzt[:, gi, m * 128:(m + 1) * 128],
                                         sb_zk[:, gi, nsl(j)],
                                         start=False, stop=(gi == NZK - 1))
                    nc.scalar.activation(sb_g1[:, m, nsl(j)], ps[:, :min(512, bc)],
                                         AF.Relu, bias=sb_tb0[:, m:m + 1])
            for m in range(2):
                for j in range(NHN):
                    ps = pmm.tile([128, 512], F32, tag="ps")
                    for kk in range(4):
                        nc.tensor.matmul(ps[:, :min(512, bc)],
                                         sb_tw1t[:, kk, m * 128:(m + 1) * 128],
                                         sb_g1[:, kk, nsl(j)],
                                         start=(kk == 0), stop=(kk == 3))
                    nc.scalar.activation(sb_g2[:, m, nsl(j)], ps[:, :min(512, bc)],
                                         AF.Relu, bias=sb_tb1[:, m:m + 1])
            for j in range(NHN):
                ps = pmm.tile([128, 512], F32, tag="ps")
                for kk in range(2):
                    nc.tensor.matmul(ps[:1, :min(512, bc)], sb_tw2t[:, kk, :],
                                     sb_g2[:, kk, nsl(j)],
                                     start=(kk == 0), stop=(kk == 1))
                nc.scalar.activation(sb_out[:, nsl(j)], ps[:1, :min(512, bc)],
                                     AF.Sigmoid, bias=sb_tb2[:, :1])

            nc.sync.dma_start(out=d_out[:], in_=sb_out[:])

    nc.compile()
    return nc


def prepare_inputs(vocab, bc, dense_x, lS_i, Wemb,
                   bW0, bb0, bW1, bb1, bW2, bb2,
                   tW0, tb0, tW1, tb1, tW2, tb2):
    """Host-side prep: shard + lay out numpy arrays exactly as the SBUF wants."""
    bf = ml_dtypes.bfloat16
    ncores = dense_x.shape[0] // bc
    wemb = np.ascontiguousarray(Wemb.reshape(N_TABLES * vocab, D)).astype(bf)

    # symmetrized top-L0 interaction weight: [108, 7, 512]
    wz = np.zeros((N_FEAT, N_FEAT, 512), np.float32)
    for p in range(len(_LI)):
        wz[_LI[p], _LJ[p], :] = 0.5 * tW0[:, D + p]
        wz[_LJ[p], _LI[p], :] = 0.5 * tW0[:, D + p]
    NZK = 7
    wzt = np.zeros((108, NZK, 512), np.float32)
    for gi in range(NZK):
        ni = min(4, N_FEAT - 4 * gi)
        blk = wz[4 * gi:4 * gi + ni].reshape(ni * 27, 512)
        wzt[:ni * 27, gi, :] = blk

    shared = dict(
        wemb=wemb,
        bw0t=np.ascontiguousarray(bW0.T).astype(bf),
        bb0c=np.ascontiguousarray(bb0.reshape(4, 128).T).astype(np.float32),
        bw1t=np.ascontiguousarray(bW1.T.reshape(4, 128, 256).transpose(1, 0, 2)).astype(bf),
        bb1c=np.ascontiguousarray(bb1.reshape(2, 128).T).astype(np.float32),
        bw2t=np.ascontiguousarray(bW2.T.reshape(2, 128, 64).transpose(1, 0, 2)).astype(bf),
        bb2c=np.ascontiguousarray(bb2.reshape(64, 1)).astype(np.float32),
        tw0xt=np.ascontiguousarray(tW0[:, :D].T).astype(bf),
        wzt=wzt.astype(bf),
        tb0c=np.ascontiguousarray(tb0.reshape(4, 128).T).astype(np.float32),
        tw1t=np.ascontiguousarray(tW1.T.reshape(4, 128, 256).transpose(1, 0, 2)).astype(bf),
        tb1c=np.ascontiguousarray(tb1.reshape(2, 128).T).astype(np.float32),
        tw2t=np.ascontiguousarray(tW2.T.reshape(2, 128, 1).transpose(1, 0, 2)).astype(bf),
        tb2c=np.array([[tb2[0]]], np.float32),
    )

    # flat row ids: table t's rows live at [t*vocab, (t+1)*vocab)
    flat_idx = (lS_i.astype(np.int64)
                + (np.arange(N_TABLES, dtype=np.int64) * vocab)[:, None])
    flat_idx = flat_idx.astype(np.int32)  # [26, B]

    in_maps = []
    T = bc // 128
    for c in range(ncores):
        sl = slice(c * bc, (c + 1) * bc)
        # idx[p, k*26+t] = flat_idx[t, c*bc + k*128 + p]
        blk = flat_idx[:, sl].reshape(N_TABLES, T, 128)  # [t, k, p]
        idx = np.ascontiguousarray(blk.transpose(2, 1, 0).reshape(128, T * N_TABLES))
        in_maps.append(dict(
            shared,
            idx=idx,
            xt13=np.ascontiguousarray(dense_x[sl].T).astype(bf),
        ))
    return in_maps


_CACHED = {}


def kernel(dense_x, lS_i, lS_o, Wemb,
           bW0, bb0, bW1, bb1, bW2, bb2,
           tW0, tb0, tW1, tb1, tW2, tb2):
    del lS_o  # offsets are arange(B): one index per bag
    vocab = Wemb.shape[1]
    B = dense_x.shape[0]
    bc = B // N_CORES
    in_maps = prepare_inputs(vocab, bc, np.asarray(dense_x), np.asarray(lS_i),
                             np.asarray(Wemb),
                             np.asarray(bW0), np.asarray(bb0),
                             np.asarray(bW1), np.asarray(bb1),
                             np.asarray(bW2), np.asarray(bb2),
                             np.asarray(tW0), np.asarray(tb0),
                             np.asarray(tW1), np.asarray(tb1),
                             np.asarray(tW2), np.asarray(tb2))
    key = (vocab, bc)
    if key not in _CACHED:
        nc = build(vocab, bc)
        m = nc.m
        nc.m = get_hw_module(m)
        _CACHED[key] = nc
    nc = _CACHED[key]
    res = bass_utils.run_bass_kernel_spmd(nc, in_maps,
                                          core_ids=list(range(N_CORES)))
    out = np.concatenate([r["out"] for r in res.results], axis=1).T  # [B, 1]
    return np.ascontiguousarray(out.astype(np.float32))
